# revision 17
# baseline (speedup 1.0000x reference)
"""Trainium2 Bass kernel for MultiHeadAttention (B=2, S=2048, D=1024, H=16).

Sharding: 8 cores = 2 (batch) x 4 (head groups of 4 heads / 256 proj cols).
Each core computes attention for its batch + head group and a partial
output projection [S, D]; host sums the 4 partials per batch and adds bo.

v2 pipeline (fp8 e4m3 attention via DoubleRow matmuls):
  1. Q/K projected with bf16 PE matmuls; DVE bias-add writes fp8 SBUF
     tiles qT8/kT8 [128, 2, S] laid out so partition 32h+r holds head h
     dim 32v+r at half index v (host pre-permutes W/bias columns).
     V projected to fp8 vaug8 [128, NT, HPG, HD+1] with ones column.
  2. Scores: per head a DoubleRow matmul contracts the 2x32 dim pairs
     -> S.T block [sk, sq] in f32 PSUM; exp (scale 1/8) on Act writes
     fp8 pt tiles; causal/pad masking = Pool multiply by 0/1 fp8 mask
     slots after exp (plus Pool memsets for j-pair range gaps).
  3. PV: DoubleRow over j-tile PAIRS (lhsT = vaug8 pair, rhs = pt pair)
     accumulating [65, sq] (row 64 = softmax denominator).
  4. Normalize (DVE recip + PE ones-broadcast + DVE mul) into f32r z.T;
     out-proj per s-tile in f32r, DMA out.
PE stream is software-pipelined: PV lags S.T by one pair, and chunk
c-1's out-proj matmuls fill PE waits inside chunk c's attention.
"""

import math
import os
import sys

import numpy as np

sys.path.insert(0, "/opt/trn_rl_repo")
sys.path.insert(0, "/opt/trn_rl_repo/concourse")

B, S, D, H = 2, 2048, 1024, 16
HD = D // H  # 64
G = 4  # head groups (cores per batch)
OG = D // G  # 256 proj cols per core
HPG = H // G  # 4 heads per core
P = 128
NT = S // P  # 16 s-tiles
CH = 512  # sq chunk width
NCH = S // CH  # 4 chunks
KT = D // P  # 8 contraction tiles for projections

_cache = {}


def _attention_structure(mask, key_padding_mask):
    """Derive block structure + per-core fp8 mask-slot data.

    Returns (struct, slot_data) where struct is hashable codegen metadata
    and slot_data[b] is an [NSLOT, P, P] float32 0/1 keep-mask array
    (transposed to [sk, sq]) for batch b.
    """
    mask = np.asarray(mask)
    kpm = np.asarray(key_padding_mask)
    full = np.zeros((B, NT, NT), dtype=bool)
    anym = np.zeros((B, NT, NT), dtype=bool)
    blocks = {}
    for b in range(B):
        for i in range(NT):
            mrow = mask[i * P:(i + 1) * P]
            for j in range(NT):
                mb = mrow[:, j * P:(j + 1) * P] | kpm[b, None, j * P:(j + 1) * P]
                full[b, i, j] = mb.all()
                anym[b, i, j] = mb.any()
                blocks[(b, i, j)] = mb
    process = (~full).any(axis=0)
    needs_mask = process & anym.any(axis=0)

    slot_map = {}
    for i in range(NT):
        for j in range(NT):
            if needs_mask[i, j]:
                slot_map[(i, j)] = len(slot_map)
    nslot = max(1, len(slot_map))

    slot_data = []
    for b in range(B):
        d = np.ones((nslot, P, P), np.float32)
        for (i, j), s in slot_map.items():
            if anym[b, i, j]:
                d[s] = (~blocks[(b, i, j)]).T.astype(np.float32)
        slot_data.append(d)

    # per-chunk j pairs
    chunks = []
    for c in range(NCH):
        tiles_i = list(range(c * 4, c * 4 + 4))
        jinfo = []
        for j in range(NT):
            ii = [i for i in tiles_i if process[i, j]]
            if ii:
                ops = tuple((i - c * 4, slot_map[(i, j)])
                            for i in ii if needs_mask[i, j])
                jinfo.append((j, min(ii) - c * 4, max(ii) - c * 4 + 1, ops))
        pairs = []
        k = 0
        while k < len(jinfo):
            grp = jinfo[k:k + 2]
            lo = min(g[1] for g in grp)
            hi = max(g[2] for g in grp)
            pairs.append((tuple(grp), lo, hi))
            k += 2
        chunks.append(tuple(pairs))
    struct = (tuple(chunks), nslot)
    return struct, slot_data


def _build_bass(struct):
    """Trace the Tile kernel from hashable structure metadata.

    Emission interleaves projection trios (K_c, Q_c, V_c) with attention
    chunk c-1 so the Act engine's exp stream starts ~8us into the kernel
    instead of after the whole projection phase.
    """
    import concourse.bass as bass
    import concourse.tile as tile
    from concourse import bacc, mybir

    chunks, nslot = struct
    f32 = mybir.dt.float32
    f32r = mybir.dt.float32r
    bf16 = mybir.dt.bfloat16
    fp8 = mybir.dt.float8e4
    DR = mybir.MatmulPerfMode.DoubleRow
    nc = bacc.Bacc("TRN2", target_bir_lowering=False, debug=False,
                   enable_asserts=False)

    xqT = nc.dram_tensor("xqT", [D, S], fp8, kind="ExternalInput").ap()
    xkT = nc.dram_tensor("xkT", [D, S], fp8, kind="ExternalInput").ap()
    xvT = nc.dram_tensor("xvT", [D, S], bf16, kind="ExternalInput").ap()
    wqT = nc.dram_tensor("wqT", [D, OG], fp8, kind="ExternalInput").ap()
    wkT = nc.dram_tensor("wkT", [D, OG], fp8, kind="ExternalInput").ap()
    wvT = nc.dram_tensor("wvT", [D, OG], bf16, kind="ExternalInput").ap()
    woT = nc.dram_tensor("woT", [OG, D], f32r, kind="ExternalInput").ap()
    bq = nc.dram_tensor("bq", [OG], f32, kind="ExternalInput").ap()
    bk = nc.dram_tensor("bk", [OG], f32, kind="ExternalInput").ap()
    bv = nc.dram_tensor("bv", [OG], f32, kind="ExternalInput").ap()
    mask8 = nc.dram_tensor("mask8", [nslot, P, P], bf16,
                           kind="ExternalInput").ap()
    out = nc.dram_tensor("out", [S, D], bf16, kind="ExternalOutput").ap()

    xqTr = xqT.rearrange("(t p) s -> p t s", p=P)
    xkTr = xkT.rearrange("(t p) s -> p t s", p=P)
    xvTr = xvT.rearrange("(t p) s -> p t s", p=P)

    with tile.TileContext(nc) as tc:
        with tc.tile_pool(name="persist", bufs=1) as persist, \
             tc.tile_pool(name="const", bufs=1) as const:
            qT8 = persist.tile([P, 2, S], fp8)       # [32h+r, half v, s]
            kT8 = persist.tile([P, 2, S], fp8)
            vaug = persist.tile([P, NT, HPG, HD + 1], bf16)
            zt01 = persist.tile([P, S], f32r)        # heads 0,1 Z.T scaled
            zt23 = persist.tile([P, S], f32r)
            woT_sb = persist.tile([P, 2, D], f32r)
            mask_sb = persist.tile([P, nslot, P], bf16)

            ones_row = const.tile([1, P], f32r)
            nc.vector.memset(ones_row.bitcast(mybir.dt.uint32), 0x3F800000)
            # E[hh, p] = 1 if p in head-half hh: bc = E.T @ R broadcasts
            # each head's reciprocal row onto its 64-partition range.
            Rrec = const.tile([33, CH], f32r)
            nc.vector.memset(Rrec.bitcast(mybir.dt.uint32), 0)
            Eind = const.tile([33, P], f32r)
            nc.vector.memset(Eind.bitcast(mybir.dt.uint32), 0)
            nc.vector.memset(Eind[0:1, 0:HD].bitcast(mybir.dt.uint32),
                             0x3F800000)
            nc.vector.memset(Eind[32:33, HD:P].bitcast(mybir.dt.uint32),
                             0x3F800000)
            bqs = const.tile([P, 2], f32)
            bks = const.tile([P, 2], f32)
            bvb = const.tile([P, OG], f32)
            warm = const.tile([1, 1], f32)
            # ones column of vaug (bf16 1.0)
            nc.vector.memset(
                vaug[:, :, :, HD:HD + 1].bitcast(mybir.dt.uint16), 0x3F80)

            # ---- Flat pools for the whole kernel ----
            xTp = tc.alloc_tile_pool(name="xT", bufs=3)
            wsb = tc.alloc_tile_pool(name="wsb", bufs=1)
            psum = tc.alloc_tile_pool(name="psum", bufs=1, space="PSUM")
            ptp = tc.alloc_tile_pool(name="pt", bufs=4)
            small = tc.alloc_tile_pool(name="small", bufs=4)
            osb = tc.alloc_tile_pool(name="osb", bufs=3)

            wqT_sb = wsb.tile([P, KT, OG], fp8, tag="w")
            wkT_sb = wsb.tile([P, KT, OG], fp8, tag="w2")
            wvT_sb = wsb.tile([P, KT, OG], bf16, tag="w3")
            wkr = wkT.rearrange("(t p) o -> p t o", p=P)
            srcs = {0: (xkTr, wkT_sb), 1: (xvTr, wvT_sb), 2: (xqTr, wqT_sb)}

            def emit_proj_step(which, c, step):
                if step == 0:   # biases first (K0's bias-add reads bks)
                    nc.sync.dma_start(bks, bk.rearrange("(t p) -> p t", p=P))
                    nc.sync.dma_start(bqs, bq.rearrange("(t p) -> p t", p=P))
                    nc.sync.dma_start(
                        bvb, bass.AP(tensor=bv.tensor, offset=bv.offset,
                                     ap=[[0, P]] + list(bv.ap)))
                    nc.scalar.activation(warm, bqs[0:1, 0:1],
                                         mybir.ActivationFunctionType.Exp,
                                         scale=1.0)
                elif step == 1:   # Q0: V weights
                    nc.sync.dma_start(
                        wvT_sb, wvT.rearrange("(t p) o -> p t o", p=P))
                elif step == 2:   # V0: mask slots
                    nc.sync.dma_start(mask_sb,
                                      mask8.rearrange("n p q -> p n q"))
                elif step == 3:   # K1: out-proj weights
                    nc.sync.dma_start(
                        woT_sb, woT.rearrange("(t p) d -> p t d", p=P))
                xr, w_sb = srcs[which]
                dt_x = bf16 if which == 1 else fp8
                xTc = xTp.tile([P, KT, CH], dt_x,
                               tag=("xTv" if which == 1 else "xT8"),
                               name="xTc")
                for kg in range(0, KT, 2):
                    if step == 0 and kg in (0, 4):
                        h = 0 if kg == 0 else 1
                        nc.sync.dma_start(
                            wkT_sb[:, h * (KT // 2):(h + 1) * (KT // 2), :],
                            wkr[:, h * (KT // 2):(h + 1) * (KT // 2), :])
                    nc.sync.dma_start(
                        xTc[:, kg:kg + 2, :],
                        xr[:, kg:kg + 2, c * CH:(c + 1) * CH])
                if step == 0:
                    nc.sync.dma_start(
                        wqT_sb, wqT.rearrange("(t p) o -> p t o", p=P))
                if which == 1:
                    return xTc
                if which != 1:
                    # K.T / Q.T halves -> fp8 [32h+r, v, s]
                    # (fp8 DoubleRow: contract k-tile pairs at 0.5 cyc/col)
                    dst = kT8 if which == 0 else qT8
                    bias_ap = bks if which == 0 else bqs
                    for v in range(2):
                        ps = psum.tile([P, CH], f32, tag="ps512",
                                       bufs=2, name="ps")
                        for kp in range(0, KT, 2):
                            nc.tensor.matmul(
                                ps, w_sb[:, kp:kp + 2, v * P:(v + 1) * P],
                                xTc[:, kp:kp + 2, :],
                                start=(kp == 0), stop=(kp == KT - 2),
                                perf_mode=DR)
                        nc.vector.tensor_scalar_add(
                            dst[:, v, c * CH:(c + 1) * CH], ps,
                            bias_ap[:, v:v + 1])
                return None

            def make_vquantum(xTc, c, st):
                # one s-tile of the V projection: 8 matmuls + bias-add
                def emit():
                    ps = psum.tile([P, OG], f32, tag="ps512",
                                   bufs=2, name="ps")
                    for k in range(KT):
                        nc.tensor.matmul(
                            ps, xTc[:, k, st * P:(st + 1) * P],
                            wvT_sb[:, k, :],
                            start=(k == 0), stop=(k == KT - 1))
                    nc.vector.tensor_add(
                        vaug[:, c * 4 + st, :, 0:HD],
                        ps.rearrange("p (h d) -> p h d", h=HPG),
                        bvb.rearrange("p (h d) -> p h d", h=HPG))
                return emit

            # ---- fills: FIFO of deferred emit closures ----
            # vfills: V-projection quanta (priority; guarded before PV use)
            fills = []
            vfills = []   # (vaug_tile_index, closure)
            vaug_ready = [-1]

            def pop_vfill():
                t, cl = vfills.pop(0)
                cl()
                vaug_ready[0] = t

            def ensure_vaug(tile_idx):
                while vaug_ready[0] < tile_idx and vfills:
                    pop_vfill()

            def emit_fill(n=1):
                for _ in range(n):
                    if vfills:
                        pop_vfill()
                    elif fills:
                        fills.pop(0)()

            def make_outproj(sg, nchunk, copy_on_act):
                def emit():
                    ob = osb.tile([P, CH], bf16, tag="ob", name="ob")
                    ps = psum.tile([P, CH], f32, tag="ps512",
                                   bufs=2, name="ps")
                    for k, zsrc in enumerate((zt01, zt23)):
                        nc.tensor.matmul(
                            ps, zsrc[:, sg * P:(sg + 1) * P],
                            woT_sb[:, k, nchunk * CH:(nchunk + 1) * CH],
                            start=(k == 0), stop=(k == 1))
                    if copy_on_act:
                        nc.scalar.copy(ob, ps)
                    else:
                        nc.vector.tensor_copy(ob, ps)
                    nc.sync.dma_start(
                        out[sg * P:(sg + 1) * P,
                            nchunk * CH:(nchunk + 1) * CH], ob)
                return emit

            def make_epilogue(c, ztaus, h0, h1):
                R = {}

                def dve1():
                    with nc.allow_low_precision(reason="fp22 recip"):
                        nc.vector.reciprocal(Rrec[0:1, :],
                                             ztaus[h0][HD:HD + 1, :])
                        nc.vector.reciprocal(Rrec[32:33, :],
                                             ztaus[h1][HD:HD + 1, :])

                def pe_bc():
                    bc = psum.tile([P, CH], f32, tag="ps512", bufs=2,
                                   name="bc")
                    nc.tensor.matmul(bc, Eind, Rrec, start=True, stop=True)
                    R["bc"] = bc

                def dve2():
                    bcs = small.tile([P, CH], f32, tag="bcs", name="bcs")
                    nc.vector.tensor_copy(bcs, R["bc"])
                    for hh, h in enumerate((h0, h1)):
                        zdst = zt01 if h < 2 else zt23
                        zpo = (h % 2) * HD
                        nc.vector.tensor_mul(
                            zdst[zpo:zpo + HD, c * CH:(c + 1) * CH],
                            ztaus[h][0:HD, :], bcs[hh * HD:hh * HD + HD, :])

                return dve1, pe_bc, dve2

            # ---- Interleaved schedule: trio(c) then attention(c) ----
            step = 0
            pending_epi = [None]
            for c in range(NCH):
                for which in (0, 2, 1):   # K_c, Q_c, V_c
                    xv = emit_proj_step(which, c, step)
                    step += 1
                    if xv is not None:
                        if c == 0:
                            for st in range(CH // P):
                                make_vquantum(xv, c, st)()
                            vaug_ready[0] = 3
                        else:
                            for st in range(CH // P):
                                vfills.append((c * 4 + st,
                                               make_vquantum(xv, c, st)))
                pairs = chunks[c]
                for hp in range(2):
                    h0, h1 = 2 * hp, 2 * hp + 1
                    ztaus = {}
                    for h in (h0, h1):
                        ztaus[h] = psum.tile([HD + 1, CH], f32,
                                             tag=f"zt{h % 2}", bufs=1,
                                             name=f"ztau{h % 2}")
                    if pending_epi[0]:
                        pending_epi[0][0]()   # recips of previous hp
                    pend = []  # pending PVs, emitted with lag 2
                    for pi, (grp, lo_p, hi_p) in enumerate(pairs):
                        pt = ptp.tile([P, 2, 2, CH], bf16, tag="pt",
                                      name="pt")
                        for jj, (j, lo, hi, ops) in enumerate(grp):
                            st_ = psum.tile([P, 2, CH], f32, tag="st",
                                            bufs=2, name="st_")
                            off, w = lo * P, (hi - lo) * P
                            for hh, h in enumerate((h0, h1)):
                                po = 32 * h
                                nc.tensor.matmul(
                                    st_[:, hh, off:off + w],
                                    kT8[po:po + 32, :, j * P:(j + 1) * P],
                                    qT8[po:po + 32, :,
                                        c * CH + off:c * CH + off + w],
                                    start=True, stop=True, perf_mode=DR,
                                    tile_position=(po, 0))
                            nc.scalar.activation(
                                pt[:, :, jj, off:off + w],
                                st_[:, :, off:off + w],
                                mybir.ActivationFunctionType.Exp,
                                scale=1.0 / math.sqrt(HD))
                            for i_rel, slot in ops:
                                so = i_rel * P
                                mb = mask_sb[:, slot, :]
                                mb2 = bass.AP(
                                    tensor=mb.tensor, offset=mb.offset,
                                    ap=[mb.ap[0], [0, 2]] + list(mb.ap[1:]))
                                nc.gpsimd.tensor_mul(
                                    pt[:, :, jj, so:so + P],
                                    pt[:, :, jj, so:so + P], mb2)
                        if pi == 0 and pending_epi[0]:
                            pending_epi[0][1]()   # bc matmul of previous hp
                        elif pi == 1 and pending_epi[0]:
                            pending_epi[0][2]()   # bcs+zmul of previous hp
                            pending_epi[0] = None
                        if len(pend) >= 2:
                            prev = pend.pop(0)
                            ensure_vaug(max(j for j, *_ in prev[1]))
                            _emit_pv(nc, ztaus, vaug, prev, h0, h1,
                                     first=(prev[2] == 0), last=False)
                            if pi >= 2:
                                emit_fill(2)
                        pend.append((pt, grp, pi))
                    while pend:
                        prev = pend.pop(0)
                        ensure_vaug(max(j for j, *_ in prev[1]))
                        _emit_pv(nc, ztaus, vaug, prev, h0, h1,
                                 first=(prev[2] == 0), last=(not pend))
                    if pending_epi[0]:   # short chunks: flush leftover
                        for f_ in pending_epi[0][1:]:
                            if f_ is not None:
                                f_()
                        pending_epi[0] = None
                    pending_epi[0] = make_epilogue(c, ztaus, h0, h1)
                if c == NCH - 1:
                    for f_ in pending_epi[0]:
                        f_()
                    pending_epi[0] = None
                    emit_fill(len(fills) + len(vfills))
                for sg in range(c * 4, c * 4 + 4):
                    for nchunk in range(2):
                        fills.append(make_outproj(
                            sg, nchunk,
                            copy_on_act=(c == NCH - 1 and (sg + nchunk) % 2 == 0)))
            emit_fill(len(fills) + len(vfills))
            for pool_ in (osb, small, ptp, psum, wsb, xTp):
                pool_.release()
    nc.compile()
    # Belt-and-braces: any write-only preamble registers that survive DCE
    # but never get ids from alloc_regs would fail walrus birverifier
    # (reg_id == -1). They are write-only, so engine-unique ids are safe;
    # keep _lo/_hi pairs adjacent and even-aligned.
    from collections import defaultdict
    ctr = defaultdict(int)
    for f_ in nc.m.functions:
        for a in f_.allocations:
            if isinstance(a, mybir.Register) and a.reg_id >= 0:
                ctr[a.engine] = max(ctr[a.engine], a.reg_id + 1)
    for f_ in nc.m.functions:
        for a in f_.allocations:
            if isinstance(a, mybir.Register) and a.reg_id == -1:
                if a.name.endswith("_lo") and ctr[a.engine] % 2:
                    ctr[a.engine] += 1
                a.reg_id = ctr[a.engine]
                ctr[a.engine] += 1
    return nc


def _emit_pv(nc, ztaus, vaug, prev, h0, h1, first, last):
    """bf16 PV per j-tile over its own sq range (accumulating [65, sq])."""
    pt, grp, _ = prev
    for jj, (j, lo, hi, ops) in enumerate(grp):
        off, w = lo * P, (hi - lo) * P
        for hh, h in enumerate((h0, h1)):
            nc.tensor.matmul(
                ztaus[h][:, off:off + w],
                vaug[:, j, h, :],
                pt[:, hh, jj, off:off + w],
                start=first and jj == 0, stop=last and jj == len(grp) - 1)


def kernel(query, key, value, mask, key_padding_mask,
           Wq, bq, Wk, bk, Wv, bv, Wo, bo, _return_perf=False):
    from concourse import bass_utils

    query = np.asarray(query, np.float32)
    key_ = np.asarray(key, np.float32)
    value = np.asarray(value, np.float32)
    Wq, Wk, Wv, Wo = (np.asarray(w, np.float32) for w in (Wq, Wk, Wv, Wo))
    bq, bk, bv, bo = (np.asarray(b_, np.float32) for b_ in (bq, bk, bv, bo))

    struct, slot_data = _attention_structure(mask, key_padding_mask)
    if struct not in _cache:
        _cache[struct] = _build_bass(struct)
    nc = _cache[struct]

    import ml_dtypes
    bf = ml_dtypes.bfloat16
    e4 = ml_dtypes.float8_e4m3
    # column permutation: new col o' = 128v + 32h + r  <-  o = 64h + 32v + r
    op_ = np.arange(OG)
    perm = 64 * ((op_ % 128) // 32) + 32 * (op_ // 128) + (op_ % 32)

    xT = {}
    for b in range(B):
        xT[("q", b)] = np.ascontiguousarray(query[b].T.astype(e4))
        xT[("k", b)] = np.ascontiguousarray(key_[b].T.astype(e4))
        xT[("v", b)] = np.ascontiguousarray(value[b].T.astype(bf))
    in_maps = []
    for core in range(8):
        b, g = core // G, core % G
        sl = slice(g * OG, (g + 1) * OG)
        in_maps.append({
            "xqT": xT[("q", b)],
            "xkT": xT[("k", b)],
            "xvT": xT[("v", b)],
            "wqT": np.ascontiguousarray(Wq[sl].T[:, perm].astype(e4)),
            "wkT": np.ascontiguousarray(Wk[sl].T[:, perm].astype(e4)),
            "wvT": np.ascontiguousarray(Wv[sl].T.astype(bf)),
            "woT": np.ascontiguousarray(Wo[:, sl].T),
            "bq": np.ascontiguousarray(bq[sl][perm]),
            "bk": np.ascontiguousarray(bk[sl][perm]),
            "bv": np.ascontiguousarray(bv[sl]),
            "mask8": slot_data[b].astype(bf),
        })

    trace = bool(int(os.environ.get("KERNEL_TRACE", "0")))
    res = bass_utils.run_bass_kernel_spmd(
        nc, in_maps, core_ids=list(range(8)), trace=trace)

    out = np.zeros((B, S, D), np.float32)
    for core in range(8):
        out[core // G] += res.results[core]["out"].astype(np.float32)
    out += bo[None, None, :]
    if _return_perf:
        return out, res
    return out
